# revision 4
# baseline (speedup 1.0000x reference)
"""BERT layer forward (nn_BertLayerForDecoder) on 8 trn2 NeuronCores.

Sharding: sequence-parallel. The (B=2, S=2048) = 4096 token rows are split
into 8 slices of 512 rows; core c owns rows [r*512, (r+1)*512) of batch
b = c // 4, r = c % 4. Q/K/V projections are computed per-slice; K^T and V
are AllGathered (two collectives, K first) within each 4-core batch group so
every core can attend its 512 query rows over the full 2048 keys. Everything
else (out-proj, LN1, FFN, LN2) is row-local, so the final output is a
disjoint row-slice per core with no further communication.

Pipeline layout (v2):
  A: K-proj (own keys) -> AllGather K^T; V-proj -> AllGather V; Q-proj.
  B: attention pair-loop, ACT-bound: per 128-key chunk, scores matmul pair
     (row-packed 2 heads) -> exp on ACT (mask bias + 1/8 scale folded in)
     -> ctx accumulation (per-head PSUM banks, ones-columns in V produce the
     softmax denominators); normalization via reciprocal_approx_fast.
     FFN-up weights (all 8 MB) prefetch into SBUF during the loop.
  C: out-proj + LN1 + PE transposes.
  D: FFN up (resident weights) -> gelu -> FFN down (streamed) -> LN2.

Numerics: matmul operands bf16 (fp32 PSUM accumulation), all vector math
(softmax normalization, LayerNorm, residuals, biases) in fp32. Softmax is
computed without max-subtraction (scores are O(1) here).

Self-contained: hardcodes all shapes; only needs numpy + ml_dtypes + the
installed concourse package.
"""

import ml_dtypes
import numpy as np

import concourse.bacc as bacc
import concourse.mybir as mybir
import concourse.tile as tile
from concourse.bass_utils import run_bass_kernel_spmd
from concourse.masks import make_identity

F32 = mybir.dt.float32
BF16 = mybir.dt.bfloat16
AF = mybir.ActivationFunctionType
OP = mybir.AluOpType
NPBF = ml_dtypes.bfloat16

B, S, D, H, DH, DFF = 2, 2048, 1024, 16, 64, 4096
P = 128
NQ = 512              # query rows per core
QC = NQ // P          # 4 q-chunks
KC = D // P           # 8 d-chunks (contraction)
SC = S // P           # 16 key chunks
FC = DFF // P         # 32 dff chunks
DG = FC // 4          # 8 ffn-up column groups (512 cols each)
WG = FC // 4          # 8 ffn-down row groups (4 k-chunks each)
NP_ = H // 2          # 8 head pairs
LAG = 2               # ctx trails exp by LAG key-chunks
EPS = 1e-12
KV_HALF = D * NQ      # bf16 elements in each of KT / V gather blocks

_CACHE = {}


def _build():
    nc = bacc.Bacc()

    # activations: pre-transposed bf16 [P, KC, NQ]; query also raw fp32
    xqT = nc.declare_dram_parameter("xqT", [P, KC, NQ], BF16, isOutput=False)
    xkT = nc.declare_dram_parameter("xkT", [P, KC, NQ], BF16, isOutput=False)
    xvT = nc.declare_dram_parameter("xvT", [P, KC, NQ], BF16, isOutput=False)
    xq = nc.declare_dram_parameter("xq", [NQ, D], F32, isOutput=False)
    msk = nc.declare_dram_parameter("mask", [S], F32, isOutput=False)
    # weights: bf16, pre-tiled
    WqT = nc.declare_dram_parameter("WqT", [P, KC, D], BF16, isOutput=False)
    WkT = nc.declare_dram_parameter("WkT", [P, KC, D], BF16, isOutput=False)
    WvT = nc.declare_dram_parameter("WvT", [P, KC, D], BF16, isOutput=False)
    WoT = nc.declare_dram_parameter("WoT", [P, KC, D], BF16, isOutput=False)
    WiT = nc.declare_dram_parameter("WiT", [DG, P, KC, NQ], BF16,
                                    isOutput=False)
    WdT = nc.declare_dram_parameter("WdT", [WG, P, 4, D], BF16, isOutput=False)
    bq = nc.declare_dram_parameter("bq", [D], F32, isOutput=False)
    bk = nc.declare_dram_parameter("bk", [D], F32, isOutput=False)
    bv = nc.declare_dram_parameter("bv", [D], F32, isOutput=False)
    bo = nc.declare_dram_parameter("bo", [D], F32, isOutput=False)
    bi = nc.declare_dram_parameter("bi", [DFF], F32, isOutput=False)
    bd = nc.declare_dram_parameter("bd", [D], F32, isOutput=False)
    g1 = nc.declare_dram_parameter("ln1_g", [D], F32, isOutput=False)
    b1 = nc.declare_dram_parameter("ln1_b", [D], F32, isOutput=False)
    g2 = nc.declare_dram_parameter("ln2_g", [D], F32, isOutput=False)
    b2 = nc.declare_dram_parameter("ln2_b", [D], F32, isOutput=False)
    out = nc.declare_dram_parameter("out", [NQ, D], F32, isOutput=True)

    # collective bounce buffers (bf16)
    ktLb = nc.dram_tensor("kt_loc", [KV_HALF], BF16)
    ktAb = nc.dram_tensor("kt_all", [4, KV_HALF], BF16)
    vLb = nc.dram_tensor("v_loc", [KV_HALF], BF16)
    vAb = nc.dram_tensor("v_all", [4, KV_HALF], BF16)
    ktL = ktLb[:].rearrange("(d c) -> d c", c=NQ)
    vL = vLb[:].rearrange("(s d) -> s d", d=D)
    GROUPS = [[0, 1, 2, 3], [4, 5, 6, 7]]

    def ktA(blk):
        return ktAb[blk, :].rearrange("(d c) -> d c", c=NQ)

    def vA(blk):
        return vAb[blk, :].rearrange("(s d) -> s d", d=D)

    with tile.TileContext(nc) as tc:
        with (
            tc.tile_pool(name="const", bufs=1) as const,
            tc.tile_pool(name="persist", bufs=1) as persist,
        ):
            # ---------- small constants (resident) ----------
            mask_sb = const.tile([P, SC], F32)
            nc.sync.dma_start(mask_sb, msk.rearrange("(c p) -> p c", p=P))
            bq_p = const.tile([P, KC], F32)
            nc.sync.dma_start(bq_p, bq.rearrange("(c p) -> p c", p=P))
            bk_p = const.tile([P, KC], F32)
            nc.sync.dma_start(bk_p, bk.rearrange("(c p) -> p c", p=P))
            bi_p = const.tile([P, FC], F32)
            eps_sb = const.tile([P, 1], F32)
            nc.vector.memset(eps_sb, EPS)

            def rep_row(pool, vec, name):
                t = pool.tile([P, D], F32, tag=name, name=name)
                nc.sync.dma_start(t, vec.ap().unsqueeze(0).to_broadcast((P, D)))
                return t

            # persistent across phases B..D
            ctxT = persist.tile([P, KC, NQ], BF16)     # ctx^T (dh-pairs, q)
            wo_b = persist.tile([P, KC, D], BF16)      # Wo (loaded early)
            wiR = persist.tile([P, DG, KC, NQ], BF16)  # all of Wi (resident)

            def layernorm(pool, x_res, qc, g_r, b_r, dst_ap, sfx):
                st6 = pool.tile([P, 2, 6], F32, tag="st6" + sfx, name="st6")
                for j in range(2):
                    nc.vector.bn_stats(
                        st6[:, j, :], x_res[:, qc, j * 512:(j + 1) * 512])
                mv = pool.tile([P, 2], F32, tag="mv" + sfx, name="mv")
                nc.vector.bn_aggr(mv, st6)
                sq = pool.tile([P, 1], F32, tag="sq" + sfx, name="sq")
                nc.scalar.activation(sq, mv[:, 1:2], AF.Sqrt, bias=eps_sb)
                rstd = pool.tile([P, 1], F32, tag="rstd" + sfx, name="rstd")
                nc.vector.reciprocal(rstd, sq)
                xn = pool.tile([P, D], F32, tag="xn" + sfx, name="xn")
                nc.vector.tensor_scalar(
                    xn, x_res[:, qc, :], mv[:, 0:1], rstd,
                    OP.subtract, OP.mult)
                xg = pool.tile([P, D], F32, tag="xg" + sfx, name="xg")
                nc.vector.tensor_tensor(xg, xn, g_r, OP.mult)
                nc.gpsimd.tensor_tensor(dst_ap, xg, b_r, OP.add)

            with tc.tile_pool(name="pqt", bufs=1) as pqt:
                QT = pqt.tile([P, KC, NQ], BF16)       # Q^T, lives A..B

                # ======== phase A: K-proj+gather, V-proj+gather, Q ========
                with (
                    tc.tile_pool(name="xT", bufs=2) as xT,
                    tc.tile_pool(name="wfullA", bufs=2) as wfullA,
                    tc.tile_pool(name="epA", bufs=1) as epA,
                    tc.tile_pool(name="psA", bufs=4, space="PSUM") as psA,
                ):
                    # K^T (own 512 keys) -> gather ASAP
                    keyT = xT.tile([P, KC, NQ], BF16, tag="xpt", name="keyT")
                    for hh in range(2):
                        nc.sync.dma_start(keyT[:, hh * 4:(hh + 1) * 4, :],
                                          xkT[:, hh * 4:(hh + 1) * 4, :])
                    wk_b = wfullA.tile([P, KC, D], BF16, tag="wfull",
                                       name="wk_b")
                    for hh in range(2):
                        nc.sync.dma_start(wk_b[:, hh * 4:(hh + 1) * 4, :],
                                          WkT[:, hh * 4:(hh + 1) * 4, :])
                    # pre-warm the exp table set while the PE works
                    expw = epA.tile([P, 1], F32, tag="expw", name="expw")
                    nc.scalar.activation(expw, eps_sb, AF.Exp)

                    for dout in range(KC):
                        kpp = psA.tile([P, NQ], F32, tag="ppA", name="kpp")
                        for kc in range(KC):
                            nc.tensor.matmul(
                                kpp, wk_b[:, kc, dout * P:(dout + 1) * P],
                                keyT[:, kc, :],
                                start=(kc == 0), stop=(kc == KC - 1))
                        kt_o = epA.tile([P, NQ], BF16, tag="kt_o",
                                        name="kt_o", bufs=4)
                        nc.vector.tensor_scalar_add(kt_o, kpp,
                                                    bk_p[:, dout:dout + 1])
                        nc.sync.dma_start(ktL[dout * P:(dout + 1) * P, :],
                                          kt_o)
                    nc.gpsimd.collective_compute(
                        "AllGather", OP.bypass, replica_groups=GROUPS,
                        ins=[ktLb[:]], outs=[ktAb[:, :]])

                    # V = value @ Wv -> v_loc, then gather
                    valT = xT.tile([P, KC, NQ], BF16, tag="xpt", name="valT")
                    for hh in range(2):
                        nc.sync.dma_start(valT[:, hh * 4:(hh + 1) * 4, :],
                                          xvT[:, hh * 4:(hh + 1) * 4, :])
                    wv_b = wfullA.tile([P, KC, D], BF16, tag="wfull",
                                       name="wv_b")
                    for hh in range(2):
                        nc.sync.dma_start(wv_b[:, hh * 4:(hh + 1) * 4, :],
                                          WvT[:, hh * 4:(hh + 1) * 4, :])
                    bv_r = rep_row(epA, bv, "bv_r")
                    vL_v = vL.rearrange("(c p) d -> p c d", p=P)
                    for sc4 in range(QC):
                        for hf in range(2):
                            pp = psA.tile([P, NQ], F32, tag="ppA", name="pp")
                            for kc in range(KC):
                                nc.tensor.matmul(
                                    pp, valT[:, kc, sc4 * P:(sc4 + 1) * P],
                                    wv_b[:, kc, hf * 512:(hf + 1) * 512],
                                    start=(kc == 0), stop=(kc == KC - 1))
                            v_o = epA.tile([P, NQ], BF16, tag="v_o",
                                           name="v_o", bufs=4)
                            nc.vector.tensor_tensor(
                                v_o, pp, bv_r[:, hf * 512:(hf + 1) * 512],
                                OP.add)
                            nc.scalar.dma_start(
                                vL_v[:, sc4, hf * 512:(hf + 1) * 512], v_o)
                    nc.gpsimd.collective_compute(
                        "AllGather", OP.bypass, replica_groups=GROUPS,
                        ins=[vLb[:]], outs=[vAb[:, :]])

                    # Q^T = Wq^T @ query^T
                    qryT = xT.tile([P, KC, NQ], BF16, tag="xpt", name="qryT")
                    for hh in range(2):
                        nc.sync.dma_start(qryT[:, hh * 4:(hh + 1) * 4, :],
                                          xqT[:, hh * 4:(hh + 1) * 4, :])
                    wq_b = wfullA.tile([P, KC, D], BF16, tag="wfull",
                                       name="wq_b")
                    for hh in range(2):
                        nc.sync.dma_start(wq_b[:, hh * 4:(hh + 1) * 4, :],
                                          WqT[:, hh * 4:(hh + 1) * 4, :])
                    for dc in range(KC):
                        pp = psA.tile([P, NQ], F32, tag="ppA", name="pp")
                        for kc in range(KC):
                            nc.tensor.matmul(
                                pp, wq_b[:, kc, dc * P:(dc + 1) * P],
                                qryT[:, kc, :],
                                start=(kc == 0), stop=(kc == KC - 1))
                        nc.vector.tensor_scalar_add(
                            QT[:, dc, :], pp, bq_p[:, dc:dc + 1])
                    # Wo for phase C (low-priority load)
                    for hh in range(2):
                        nc.sync.dma_start(wo_b[:, hh * 4:(hh + 1) * 4, :],
                                          WoT[:, hh * 4:(hh + 1) * 4, :])

                # ======== phase B: attention ========
                with (
                    tc.tile_pool(name="vsb", bufs=1) as vsb,
                    tc.tile_pool(name="vstr", bufs=3) as vstr,
                    tc.tile_pool(name="ktp", bufs=3) as ktp,
                    tc.tile_pool(name="probsp", bufs=4) as probsp,
                    tc.tile_pool(name="smallB", bufs=2) as smallB,
                    tc.tile_pool(name="ps_sc", bufs=2, space="PSUM") as ps_sc,
                    tc.tile_pool(name="ps_cv", bufs=4, space="PSUM") as ps_cv,
                ):
                    pkt_tiles = {}

                    def prefetch_pkt(p):
                        t = ktp.tile([P, S], BF16, tag="pkt", name="pkt")
                        for blk in range(4):
                            nc.gpsimd.dma_start(
                                t[:, blk * NQ:(blk + 1) * NQ],
                                ktA(blk)[p * P:(p + 1) * P, :])
                        pkt_tiles[p] = t

                    for p in range(3):
                        prefetch_pkt(p)

                    # V + ones cols, built from the gather
                    Vs = vsb.tile([P, SC, H, DH + 2], BF16)
                    nc.gpsimd.memset(Vs[:, :, :, DH:DH + 2], 1.0)
                    for blk in range(4):
                        for c in range(QC):
                            vt = vstr.tile([P, D], BF16, tag="vstr",
                                           name="vt")
                            nc.sync.dma_start(vt,
                                              vA(blk)[c * P:(c + 1) * P, :])
                            nc.gpsimd.tensor_copy(
                                Vs[:, blk * QC + c, :, 0:DH],
                                vt.rearrange("p (h dh) -> p h dh", dh=DH))

                    # prefetch all FFN-up weights + later-needed small consts
                    for dg in range(DG):
                        nc.sync.dma_start(wiR[:, dg, :, :], WiT[dg, :, :, :])
                    nc.sync.dma_start(bi_p, bi.rearrange("(c p) -> p c", p=P))

                    cps = {}
                    ring = []
                    for idx in range(NP_ * SC + LAG):
                        pair, sc = divmod(idx, SC)
                        if idx < NP_ * SC:
                            if sc == 0:
                                cps[pair] = (
                                    ps_cv.tile([DH + 2, NQ], F32, tag="cp",
                                               name="cpA"),
                                    ps_cv.tile([DH + 2, NQ], F32, tag="cp",
                                               name="cpB"))
                                if pair + 3 < NP_:
                                    prefetch_pkt(pair + 3)
                            sp = ps_sc.tile([P, 2, NQ], F32, tag="sp",
                                            name="sp")
                            pkt = pkt_tiles[pair]
                            for i in range(2):
                                nc.tensor.matmul(
                                    sp[:, i, :],
                                    pkt[i * DH:(i + 1) * DH,
                                        sc * P:(sc + 1) * P],
                                    QT[i * DH:(i + 1) * DH, pair, :],
                                    start=True, stop=True)
                            pr = probsp.tile([P, 2, NQ], BF16, tag="probs",
                                             name="probs")
                            nc.scalar.activation(
                                pr, sp, AF.Exp,
                                bias=mask_sb[:, sc:sc + 1], scale=0.125)
                            ring.append((pr, pair, sc))
                        if idx >= LAG:
                            pr, cpair, csc = ring.pop(0)
                            cpA, cpB = cps[cpair]
                            for i, cp in ((0, cpA), (1, cpB)):
                                nc.tensor.matmul(
                                    cp, Vs[:, csc, 2 * cpair + i, :],
                                    pr[:, i, :],
                                    start=(csc == 0), stop=(csc == SC - 1))
                            if csc == SC - 1:
                                for i, cp in ((0, cpA), (1, cpB)):
                                    dnm = smallB.tile([1, NQ], F32, tag="dnm",
                                                      name="dnm")
                                    nc.vector.tensor_copy(dnm,
                                                          cp[DH:DH + 1, :])
                                    rcp = smallB.tile([1, NQ], F32, tag="rcp",
                                                      name="rcp")
                                    nc.vector.reciprocal_approx_fast(rcp, dnm)
                                    rep = smallB.tile([DH, NQ], F32,
                                                      tag="rep", name="rep")
                                    nc.gpsimd.partition_broadcast(rep, rcp)
                                    nc.vector.tensor_tensor(
                                        ctxT[i * DH:(i + 1) * DH, cpair, :],
                                        cp[0:DH, :], rep, OP.mult)
                                del cps[cpair]
                                pkt_tiles.pop(cpair, None)

            # ======== phases C+D share the big row buffers ========
            with tc.tile_pool(name="pCD", bufs=1) as pCD:
              attn_res = pCD.tile([P, QC, D], F32)   # attn+residual
              attn1 = pCD.tile([P, QC, D], F32)      # LN1 out (residual)
              attn1T = pCD.tile([P, KC, NQ], BF16)
              # ======== phase C: out-proj + LN1 + transpose ========
              with (
                tc.tile_pool(name="qnatC", bufs=1) as qnatC,
                tc.tile_pool(name="repC", bufs=1) as repC,
                tc.tile_pool(name="lnC", bufs=2) as lnC,
                tc.tile_pool(name="a1bfC", bufs=1) as a1bfC,
                tc.tile_pool(name="identC", bufs=1) as identC,
                tc.tile_pool(name="psC", bufs=3, space="PSUM") as psC,
                tc.tile_pool(name="psT2", bufs=2, space="PSUM") as psT2,
              ):
                ident = identC.tile([P, P], BF16)
                make_identity(nc, ident)
                bo_r = rep_row(repC, bo, "bo_r")
                g1_r = rep_row(repC, g1, "g1_r")
                b1_r = rep_row(repC, b1, "b1_r")
                q_nat = qnatC.tile([P, QC, D], F32)
                xq_v = xq.rearrange("(c p) d -> p c d", p=P)
                qbo = qnatC.tile([P, QC, D], F32)
                for qc in range(QC):
                    nc.sync.dma_start(q_nat[:, qc, :], xq_v[:, qc, :])
                    nc.vector.tensor_tensor(qbo[:, qc, :], q_nat[:, qc, :],
                                            bo_r, OP.add)
                for qc in range(QC):
                    for hf in range(2):
                        pp = psC.tile([P, NQ], F32, tag="ppC", name="pp")
                        for pc_ in range(KC):
                            nc.tensor.matmul(
                                pp, ctxT[:, pc_, qc * P:(qc + 1) * P],
                                wo_b[:, pc_, hf * 512:(hf + 1) * 512],
                                start=(pc_ == 0), stop=(pc_ == KC - 1))
                        nc.vector.tensor_tensor(
                            attn_res[:, qc, hf * 512:(hf + 1) * 512], pp,
                            qbo[:, qc, hf * 512:(hf + 1) * 512], OP.add)

                attn1_bf = a1bfC.tile([P, QC, D], BF16)
                for qc in range(QC):
                    layernorm(lnC, attn_res, qc, g1_r, b1_r,
                              attn1[:, qc, :], "C")
                    nc.vector.tensor_copy(attn1_bf[:, qc, :],
                                          attn1[:, qc, :])
                    pt = psT2.tile([P, KC, P], BF16, tag="ptr2", name="pt")
                    for dc in range(KC):
                        nc.tensor.transpose(
                            pt[:, dc, :],
                            attn1_bf[:, qc, dc * P:(dc + 1) * P], ident)
                    nc.vector.tensor_copy(
                        attn1T[:, :, qc * P:(qc + 1) * P], pt)

              # ======== phase D: FFN ========
              with tc.tile_pool(name="repD", bufs=1) as repD, \
                 tc.tile_pool(name="interp", bufs=1) as interp, \
                 tc.tile_pool(name="epD", bufs=2) as epD, \
                 tc.tile_pool(name="lnD", bufs=2) as lnD:
                bd_r = rep_row(repD, bd, "bd_r")
                g2_r = rep_row(repD, g2, "g2_r")
                b2_r = rep_row(repD, b2, "b2_r")
                interT = interp.tile([P, FC, NQ], BF16)

                # D1: interT = gelu(Wi^T @ attn1^T + bi), resident weights
                with tc.tile_pool(name="psD1", bufs=2, space="PSUM") as psD1:
                    for dg in range(DG):
                        ppg = [psD1.tile([P, NQ], F32, tag=f"ppD1_{j}",
                                         name=f"ppD1_{j}")
                               for j in range(4)]
                        for kc in range(KC):
                            for j in range(4):
                                nc.tensor.matmul(
                                    ppg[j],
                                    wiR[:, dg, kc, j * P:(j + 1) * P],
                                    attn1T[:, kc, :],
                                    start=(kc == 0), stop=(kc == KC - 1))
                        for j in range(4):
                            dc = dg * 4 + j
                            nc.scalar.activation(
                                interT[:, dc, :], ppg[j], AF.Gelu,
                                bias=bi_p[:, dc:dc + 1])
                # residual base for D2 epilogue: attn1 += bd (in place)
                for qc in range(QC):
                    nc.gpsimd.tensor_tensor(attn1[:, qc, :], attn1[:, qc, :],
                                            bd_r, OP.add)

                # D2: layer_out = interT^T @ Wd + bd; +attn1; LN2
                layer_res = attn_res  # reuse buffer
                out_v = out.rearrange("(c p) d -> p c d", p=P)
                with tc.tile_pool(name="psD2", bufs=2, space="PSUM") as psD2, \
                     tc.tile_pool(name="wdD", bufs=2) as wdD:
                    for hf in range(2):
                        pps = [psD2.tile([P, NQ], F32, tag=f"ppD2_{j}",
                                         name=f"ppD2_{j}")
                               for j in range(4)]
                        for g in range(WG):
                            wd_g = wdD.tile([P, 4, NQ], BF16, tag="wd_g",
                                            name="wd_g")
                            for kk in range(4):
                                nc.sync.dma_start(
                                    wd_g[:, kk, :],
                                    WdT[g, :, kk, hf * 512:(hf + 1) * 512])
                            for k2 in range(4):
                                kc2 = g * 4 + k2
                                for qc in range(QC):
                                    nc.tensor.matmul(
                                        pps[qc],
                                        interT[:, kc2, qc * P:(qc + 1) * P],
                                        wd_g[:, k2, :],
                                        start=(kc2 == 0),
                                        stop=(kc2 == FC - 1))
                        for qc in range(QC):
                            nc.vector.tensor_tensor(
                                layer_res[:, qc, hf * 512:(hf + 1) * 512],
                                pps[qc],
                                attn1[:, qc, hf * 512:(hf + 1) * 512],
                                OP.add)
                    for qc in range(QC):
                        o_t = epD.tile([P, D], F32, tag="o_t", name="o_t")
                        layernorm(lnD, layer_res, qc, g2_r, b2_r, o_t, "D")
                        nc.sync.dma_start(out_v[:, qc, :], o_t)

    nc.compile()
    return nc


def _get_program():
    if "nc" not in _CACHE:
        _CACHE["nc"] = _build()
    return _CACHE["nc"]


def _prep_shared(inputs):
    def f32(x):
        return np.ascontiguousarray(np.asarray(x), dtype=np.float32)

    def bf(x):
        return np.ascontiguousarray(np.asarray(x, dtype=NPBF))

    Wq, Wk, Wv, Wo = (f32(inputs[n]) for n in ["Wq", "Wk", "Wv", "Wo"])
    Wi, Wd = f32(inputs["Wi"]), f32(inputs["Wd"])

    def tile_sq(w):  # [D, D] -> [P, KC, D]
        return bf(w.reshape(KC, P, D).transpose(1, 0, 2))

    shared = {
        "WqT": tile_sq(Wq), "WkT": tile_sq(Wk),
        "WvT": tile_sq(Wv), "WoT": tile_sq(Wo),
        # Wi [D, DFF] -> [DG, P, KC, NQ]: (d=kc*P+p, f=dg*NQ+j)
        "WiT": bf(Wi.reshape(KC, P, DG, NQ).transpose(2, 1, 0, 3)),
        # Wd [DFF, D] -> [WG, P, 4, D]: (f=g*NQ+k2*P+p)
        "WdT": bf(Wd.reshape(WG, 4, P, D).transpose(0, 2, 1, 3)),
    }
    for n in ["bq", "bk", "bv", "bo", "bi", "bd",
              "ln1_g", "ln1_b", "ln2_g", "ln2_b"]:
        shared[n] = f32(inputs[n])
    return shared


def _run(inputs, trace=False):
    nc = _get_program()

    def f32(x):
        return np.ascontiguousarray(np.asarray(x), dtype=np.float32)

    def pick(*names):
        for n in names:
            if n in inputs:
                return inputs[n]
        raise KeyError(names[0])

    q = f32(pick("query"))
    k = f32(pick("key_in", "key"))
    v = f32(pick("value_in", "value"))
    m = f32(pick("attention_mask", "mask"))
    shared = _prep_shared(inputs)

    def xpose_tile(x_slice):  # [NQ, D] fp32 -> [P, KC, NQ] bf16
        xT = x_slice.T.astype(NPBF)           # [D, NQ]
        return np.ascontiguousarray(
            xT.reshape(KC, P, NQ).transpose(1, 0, 2))

    in_maps = []
    for c in range(8):
        b, r = c // 4, c % 4
        sl = slice(r * NQ, (r + 1) * NQ)
        im = dict(shared)
        im["xqT"] = xpose_tile(q[b, sl])
        im["xkT"] = xpose_tile(k[b, sl])
        im["xvT"] = xpose_tile(v[b, sl])
        im["xq"] = np.ascontiguousarray(q[b, sl])
        im["mask"] = np.ascontiguousarray(m[b, 0, 0, :])
        in_maps.append(im)

    res = run_bass_kernel_spmd(nc, in_maps, core_ids=list(range(8)),
                               trace=trace)
    full = np.empty((B, S, D), dtype=np.float32)
    for c in range(8):
        b, r = c // 4, c % 4
        full[b, r * NQ:(r + 1) * NQ, :] = res.results[c]["out"]
    return full, res


def kernel(**inputs):
    full, _ = _run(inputs)
    return full
